# revision 21
# baseline (speedup 1.0000x reference)
import os
import sys

os.environ.setdefault("JAX_PLATFORMS", "")
sys.path.insert(0, "/opt/trn_rl_repo")

import numpy as np
import ml_dtypes

BF16 = ml_dtypes.bfloat16
INV = np.float32(1.0 / np.sqrt(1.0 + 1e-5))
GROUPS = [[0, 1, 2, 3], [4, 5, 6, 7]]
CHUNKS = (0, 512)

_CACHE = {}


def _blockP(a):
    a = np.ascontiguousarray(a)
    R, C = a.shape
    assert R % 128 == 0, (R, C)
    nb = R // 128
    return np.ascontiguousarray(a.reshape(nb, 128, C).transpose(1, 0, 2).reshape(128, nb * C))


# name -> (shape, "bf"/"f32")
SPECS = {
    "xT_hi": ((3, 1024), "bf"), "xT_lo": ((3, 1024), "bf"),
    "c1T": ((3, 64), "bf"),
    "c2T": ((64, 128), "bf"),
    "c3T": ((128, 256), "bf"),
    "p1T": ((128, 512), "bf"),
    "p2T": ((128, 512), "bf"),
    "fb1": ((128, 1), "f32"), "fb2": ((128, 1), "f32"), "fb3": ((128, 2), "f32"),
    "fbp1": ((128, 2), "f32"), "fbp2": ((128, 2), "f32"),
    "qkT0": ((128, 256), "f32"), "qkT1": ((128, 256), "f32"),
    "vwT0": ((128, 512), "bf"), "vwT1": ((128, 512), "bf"),
    "twT0": ((128, 512), "bf"), "twT1": ((128, 512), "bf"),
    "vbb0": ((128, 256), "f32"), "vbb1": ((128, 256), "f32"),
    "tbf0": ((128, 2), "f32"), "tbf1": ((128, 2), "f32"),
    "cfT": ((128, 4096), "bf"), "cfb": ((128, 4), "f32"),
    "s1fT": ((128, 2048), "bf"), "s1gT": ((128, 2048), "bf"),
    "s1b": ((128, 4), "f32"),
    "s2T": ((128, 1024), "bf"), "s2b": ((128, 2), "f32"),
    "s3T": ((128, 100), "bf"),
}


def _build(reps=1):
    from concourse import tile, bacc, mybir

    dt = mybir.dt
    AF = mybir.ActivationFunctionType
    AX = mybir.AxisListType
    ALU = mybir.AluOpType
    bf, f32, f8 = dt.bfloat16, dt.float32, dt.float8e4

    nc = bacc.Bacc("TRN2", target_bir_lowering=False, debug=False, num_devices=8)

    D = {}
    for nm, (sh, dn) in SPECS.items():
        D[nm] = nc.dram_tensor(nm, list(sh), bf if dn == "bf" else f32, kind="ExternalInput")
    out_d = nc.dram_tensor("out", [50, 1024], f32, kind="ExternalOutput")

    with tile.TileContext(nc) as tc:
        with (
            tc.tile_pool(name="pers", bufs=1) as pers,
            tc.tile_pool(name="dramp", bufs=1, space="DRAM") as dramp,
        ):
            def sload(pool, nm):
                sh, dn = SPECS[nm]
                t = pool.tile(list(sh), bf if dn == "bf" else f32, tag=nm, name=nm + "_sb")
                nc.sync.dma_start(t[:], D[nm][:])
                return t

            S = {}
            for nm in ("qkT0", "qkT1", "vwT0", "vwT1", "twT0", "twT1",
                       "vbb0", "vbb1", "tbf0", "tbf1"):
                S[nm] = sload(pers, nm)

            xm = pers.tile([128, 2048], f32, tag="xm")
            # bf16 layer outputs: slot[0]=front, slot[L+1]=SA layer L output
            slot = [pers.tile([128, 2048], bf, tag=f"s{s}", name=f"s{s}") for s in range(5)]
            ones1 = pers.tile([1, 128], f32, tag="ones1")
            nc.gpsimd.memset(ones1[:], 1.0)

            agq_in = dramp.tile([128, 1024], f8, tag="agq_in")
            agq_out = dramp.tile([512, 1024], f8, tag="agq_out")
            agv_in = dramp.tile([128, 2048], f8, tag="agv_in")
            agv_out = dramp.tile([512, 2048], f8, tag="agv_out")
            ar1_in = dramp.tile([128, 16], f32, tag="ar1_in")
            ar1_out = dramp.tile([128, 16], f32, tag="ar1_out")
            ar2_in = dramp.tile([128, 16], f32, tag="ar2_in")
            ar2_out = dramp.tile([128, 16], f32, tag="ar2_out")
            gm_in = dramp.tile([128, 4], f32, tag="gm_in")
            gm_out = dramp.tile([128, 4], f32, tag="gm_out")

            def emit_pipeline():
                # ---------------- front chain ----------------
                with (
                    tc.tile_pool(name="fp", bufs=1) as fp,
                    tc.tile_pool(name="fps", bufs=1, space="PSUM") as fps,
                ):
                    for nm in ("xT_hi", "xT_lo", "c1T", "c2T", "c3T", "p1T", "p2T",
                               "fb1", "fb2", "fb3", "fbp1", "fbp2"):
                        S[nm] = sload(fp, nm)

                    chain = [
                        ("c1T", 1, 64, "fb1"),
                        ("c2T", 1, 128, "fb2"),
                        ("c3T", 1, 256, "fb3"),
                        ("p1T", 2, 256, "fbp1"),
                        ("p2T", 2, 256, "fbp2"),
                    ]
                    cur = S["xT_hi"]
                    cur_lo = S["xT_lo"]
                    for li, (wn, kb, Cout, bn) in enumerate(chain):
                        wt, bt = S[wn], S[bn]
                        nob = (Cout + 127) // 128
                        P_out = min(Cout, 128)
                        last = li == len(chain) - 1
                        if last:
                            nh = slot[0]
                        else:
                            nh = fp.tile([P_out, nob * 1024], bf, tag=f"hh{li}", name=f"hh{li}")
                        for ob in range(nob):
                            Mob = min(128, Cout - 128 * ob)
                            for c0 in CHUNKS:
                                ps = fps.tile([Mob, 512], f32, tag="fps_t", bufs=4, name="fps_t")
                                tot = kb + (1 if li == 0 else 0)
                                n = 0
                                for kbi in range(kb):
                                    lh = wt[:, kbi * Cout + 128 * ob: kbi * Cout + 128 * ob + Mob]
                                    rh = cur[:, kbi * 1024 + c0: kbi * 1024 + c0 + 512]
                                    nc.tensor.matmul(ps[:], lh, rh, start=(n == 0), stop=(n == tot - 1))
                                    n += 1
                                    if li == 0:
                                        rl = cur_lo[:, kbi * 1024 + c0: kbi * 1024 + c0 + 512]
                                        nc.tensor.matmul(ps[:], lh, rl, start=False, stop=(n == tot - 1))
                                        n += 1
                                osl = slice(ob * 1024 + c0, ob * 1024 + c0 + 512)
                                if last:
                                    nc.scalar.activation(xm[0:Mob, osl], ps[:], AF.Relu,
                                                         bias=bt[0:Mob, ob: ob + 1])
                                    nc.vector.tensor_copy(nh[0:Mob, osl], xm[0:Mob, osl])
                                else:
                                    nc.scalar.activation(nh[0:Mob, osl], ps[:], AF.Relu,
                                                         bias=bt[0:Mob, ob: ob + 1])
                        cur = nh
                        cur_lo = None

                # ---------------- 4 SA layers ----------------
                with tc.tile_pool(name="sap", bufs=1) as sp:
                    for L in range(4):
                        v = 0 if L == 0 else 1
                        ih = slot[L]
                        oh = slot[L + 1]
                        qh = S[f"qkT{v}"]
                        vh = S[f"vwT{v}"]
                        th = S[f"twT{v}"]
                        vbb, tbt = S[f"vbb{v}"], S[f"tbf{v}"]

                        xq_loc = sp.tile([128, 1024], f8, tag="xq_loc", name="xq_loc")
                        xvt_loc = sp.tile([128, 2048], f8, tag="xvt_loc", name="xvt_loc")
                        xq_full = sp.tile([128, 4096], f8, tag="xq_full", name="xq_full")
                        xvt_full = sp.tile([128, 8192], f8, tag="xvt_full", name="xvt_full")
                        P_sb = sp.tile([128, 32 * 1024], f8, tag="P_sb", name="P_sb")
                        rs_a = sp.tile([128, 16], f32, tag="rs_a", name="rs_a")
                        rs_b2 = sp.tile([128, 16], f32, tag="rs_b2", name="rs_b2")
                        rg_a = sp.tile([128, 16], f32, tag="rg_a", name="rg_a")
                        rg_b = sp.tile([128, 16], f32, tag="rg_b", name="rg_b")
                        r_f = sp.tile([128, 32], f32, tag="r_f", name="r_f")
                        r_b = sp.tile([128, 32], f8, tag="r_b", name="r_b")
                        A_sb = sp.tile([128, 2048], f32, tag="A_sb", name="A_sb")
                        xr_hi = sp.tile([128, 2048], bf, tag="xr_hi", name="xr_hi")
                        icb = sp.tile([128, 1024], f32, tag="icb", name="icb")
                        cs_row = sp.tile([1, 1024], f32, tag="cs_row", name="cs_row")

                        # phase 0: local xq and xv^T; AG triggers asap
                        with tc.tile_pool(name="ps0", bufs=1, space="PSUM") as ps0:
                            for c0 in CHUNKS:
                                qps = ps0.tile([128, 512], f32, tag="qps", bufs=2, name="qps")
                                for kbi in range(2):
                                    nc.tensor.matmul(
                                        qps[:], qh[:, kbi * 128: kbi * 128 + 128],
                                        xm[:, kbi * 1024 + c0: kbi * 1024 + c0 + 512],
                                        start=(kbi == 0), stop=(kbi == 1))
                                nc.vector.tensor_copy(xq_loc[:, c0: c0 + 512], qps[:])
                            nc.sync.dma_start(agq_in[:], xq_loc[:])
                            nc.gpsimd.collective_compute(
                                "AllGather", ALU.bypass, replica_groups=GROUPS,
                                ins=[agq_in.opt()], outs=[agq_out.opt()],
                            )
                            for nb in range(8):
                                vps = ps0.tile([128, 256], f32, tag="vps", bufs=2, name="vps")
                                for kbi in range(2):
                                    nc.tensor.matmul(
                                        vps[:], ih[:, kbi * 1024 + nb * 128: kbi * 1024 + nb * 128 + 128],
                                        vh[:, kbi * 256: kbi * 256 + 256],
                                        start=(kbi == 0), stop=(kbi == 1))
                                nc.vector.tensor_add(xvt_loc[:, nb * 256: nb * 256 + 256], vps[:], vbb[:])
                            nc.sync.dma_start(agv_in[:], xvt_loc[:])
                            nc.gpsimd.collective_compute(
                                "AllGather", ALU.bypass, replica_groups=GROUPS,
                                ins=[agv_in.opt()], outs=[agv_out.opt()],
                            )
                            # A = tw @ x while the AllGathers run
                            for ob in range(2):
                                for c0 in CHUNKS:
                                    aps = ps0.tile([128, 512], f32, tag="aps", bufs=2, name="aps")
                                    for kbi in range(2):
                                        nc.tensor.matmul(
                                            aps[:], th[:, kbi * 256 + 128 * ob: kbi * 256 + 128 * ob + 128],
                                            ih[:, kbi * 1024 + c0: kbi * 1024 + c0 + 512],
                                            start=(kbi == 0), stop=(kbi == 1))
                                    nc.vector.tensor_copy(A_sb[:, ob * 1024 + c0: ob * 1024 + c0 + 512],
                                                          aps[:])
                        for k in range(4):
                            nc.sync.dma_start(xq_full[:, k * 1024: (k + 1) * 1024],
                                              agq_out[k * 128: (k + 1) * 128, :])

                        # phase 1: energy/exp pipeline (eps 2x2=4 banks) with
                        # x_r(cb0)+colsum chains (2+2 banks) overlapping the tail
                        with tc.tile_pool(name="ps1", bufs=1, space="PSUM") as ps1:
                                def emit_energy(g):
                                    eps = ps1.tile([128, 1024], f32, tag="eps", bufs=2, name="eps")
                                    lhs = xq_full[:, g * 128: (g + 1) * 128]
                                    nc.tensor.matmul(eps[:, 0:512], lhs, xq_loc[:, 0:512],
                                                     start=True, stop=True)
                                    nc.tensor.matmul(eps[:, 512:1024], lhs, xq_loc[:, 512:1024],
                                                     start=True, stop=True)
                                    rst = rs_a[:, g: g + 1] if g < 16 else rs_b2[:, g - 16: g - 15]
                                    nc.scalar.activation(P_sb[:, g * 1024: (g + 1) * 1024], eps[:],
                                                         AF.Exp, accum_out=rst)

                                xr0 = [ps1.tile([128, 512], f32, tag=f"xr0_{i}", name=f"xr0_{i}")
                                       for i in range(2)]
                                csp = [ps1.tile([1, 512], f32, tag=f"csp{ci}", name=f"csp{ci}")
                                       for ci in range(2)]

                                def emit_pass1(g):
                                    # scale xv^T rows of block g by 1/r (both
                                    # channel halves), then cb0 x_r + colsum
                                    nc.vector.tensor_scalar_mul(
                                        xvt_full[:, g * 256: (g + 1) * 256],
                                        xvt_full[:, g * 256: (g + 1) * 256], r_f[:, g: g + 1])
                                    for ci, c0 in enumerate(CHUNKS):
                                        nc.tensor.matmul(
                                            xr0[ci][:],
                                            xvt_full[:, g * 256: g * 256 + 128],
                                            P_sb[:, g * 1024 + c0: g * 1024 + c0 + 512],
                                            start=(g == 0), stop=(g == 31))
                                    for ci, c0 in enumerate(CHUNKS):
                                        nc.tensor.matmul(
                                            csp[ci][:], r_b[:, g: g + 1],
                                            P_sb[:, g * 1024 + c0: g * 1024 + c0 + 512],
                                            start=(g == 0), stop=(g == 31))

                                for g in range(16):
                                    emit_energy(g)
                                nc.scalar.dma_start(ar1_in[:], rs_a[:])
                                nc.gpsimd.collective_compute(
                                    "AllReduce", ALU.add, replica_groups=GROUPS,
                                    ins=[ar1_in.opt()], outs=[ar1_out.opt()],
                                )
                                for g in range(16, 32):
                                    emit_energy(g)
                                # xvt copies here: AG#2 has landed by now, and
                                # putting them before the AR triggers would block
                                # the in-order GpSimd queue on AG#2 completion
                                for k in range(4):
                                    nc.sync.dma_start(xvt_full[:, k * 2048: (k + 1) * 2048],
                                                      agv_out[k * 128: (k + 1) * 128, :])
                                nc.scalar.dma_start(ar2_in[:], rs_b2[:])
                                nc.gpsimd.collective_compute(
                                    "AllReduce", ALU.add, replica_groups=GROUPS,
                                    ins=[ar2_in.opt()], outs=[ar2_out.opt()],
                                )

                                nc.scalar.dma_start(rg_a[:], ar1_out[:])
                                nc.vector.reciprocal(r_f[:, 0:16], rg_a[:])
                                nc.vector.tensor_scalar_mul(r_f[:, 0:16], r_f[:, 0:16], 4096.0)
                                nc.vector.tensor_copy(r_b[:, 0:16], r_f[:, 0:16])
                                for g in range(16):
                                    emit_pass1(g)

                                nc.scalar.dma_start(rg_b[:], ar2_out[:])
                                nc.vector.reciprocal(r_f[:, 16:32], rg_b[:])
                                nc.vector.tensor_scalar_mul(r_f[:, 16:32], r_f[:, 16:32], 4096.0)
                                nc.vector.tensor_copy(r_b[:, 16:32], r_f[:, 16:32])
                                for g in range(16, 32):
                                    emit_pass1(g)

                                # colsum -> broadcast -> reciprocal (at 128
                                # partitions; a [1,N] reciprocal is serial and
                                # costs ~6.5us); icb folds into the x_r evac
                                for ci, c0 in enumerate(CHUNKS):
                                    nc.vector.tensor_copy(cs_row[:, c0: c0 + 512], csp[ci][:])
                                for ci, c0 in enumerate(CHUNKS):
                                    ibp = ps1.tile([128, 512], f32, tag=f"csp{ci}", name="ibp")
                                    nc.tensor.matmul(ibp[:], ones1[:], cs_row[:, c0: c0 + 512],
                                                     start=True, stop=True)
                                    nc.vector.reciprocal(icb[:, c0: c0 + 512], ibp[:])
                                for ci, c0 in enumerate(CHUNKS):
                                    nc.vector.tensor_mul(xr_hi[:, c0: c0 + 512], xr0[ci][:],
                                                         icb[:, c0: c0 + 512])

                                xr1 = [ps1.tile([128, 512], f32, tag="eps", bufs=2, name=f"xr1_{i}")
                                       for i in range(2)]
                                for g in range(32):
                                    for ci, c0 in enumerate(CHUNKS):
                                        nc.tensor.matmul(
                                            xr1[ci][:],
                                            xvt_full[:, g * 256 + 128: g * 256 + 256],
                                            P_sb[:, g * 1024 + c0: g * 1024 + c0 + 512],
                                            start=(g == 0), stop=(g == 31))
                                for ci, c0 in enumerate(CHUNKS):
                                    nc.vector.tensor_mul(xr_hi[:, 1024 + c0: 1024 + c0 + 512],
                                                         xr1[ci][:], icb[:, c0: c0 + 512])

                                # phase 3: B = tw@xr, y = relu(A - B*icb + tbf),
                                # residual add; c0-major so the next layer's xq
                                # chunks unblock as early as possible
                                for ci, c0 in enumerate(CHUNKS):
                                    for ob in range(2):
                                        bps = ps1.tile([128, 512], f32, tag=f"xr0_{ci}", name="bps")
                                        for kbi in range(2):
                                            nc.tensor.matmul(
                                                bps[:], th[:, kbi * 256 + 128 * ob: kbi * 256 + 128 * ob + 128],
                                                xr_hi[:, kbi * 1024 + c0: kbi * 1024 + c0 + 512],
                                                start=(kbi == 0), stop=(kbi == 1))
                                        osl = slice(ob * 1024 + c0, ob * 1024 + c0 + 512)
                                        nc.vector.tensor_sub(A_sb[:, osl], A_sb[:, osl], bps[:])
                                        yv = sp.tile([128, 512], f32, tag="scr", bufs=2, name="yv")
                                        nc.scalar.activation(yv[:], A_sb[:, osl], AF.Relu,
                                                             bias=tbt[:, ob: ob + 1])
                                        nc.vector.tensor_add(xm[:, osl], xm[:, osl], yv[:])
                                        nc.vector.tensor_copy(oh[:, osl], xm[:, osl])

                # ---------------- back end ----------------
                with tc.tile_pool(name="bp", bufs=1) as bp:
                    for nm in ("cfT", "cfb", "s1fT", "s1gT", "s1b", "s2T", "s2b", "s3T"):
                        S[nm] = sload(bp, nm)

                    face_h = bp.tile([128, 4096], bf, tag="face_h", name="face_h")
                    gml = bp.tile([128, 4], f32, tag="gml", name="gml")
                    rmx = bp.tile([128, 8], f32, tag="rmx", name="rmx")

                    with tc.tile_pool(name="psA", bufs=1, space="PSUM") as psA:
                        for ob in range(4):
                            for ci, c0 in enumerate(CHUNKS):
                                fpt = psA.tile([128, 512], f32, tag="fpsb", bufs=4, name="fpt")
                                n, tot = 0, 8
                                for sk in range(8):
                                    s, cb = 1 + sk // 2, sk % 2
                                    rh = slot[s][:, cb * 1024 + c0: cb * 1024 + c0 + 512]
                                    lh = S["cfT"][:, sk * 512 + 128 * ob: sk * 512 + 128 * ob + 128]
                                    nc.tensor.matmul(fpt[:], lh, rh, start=(n == 0), stop=(n == tot - 1))
                                    n += 1
                                nc.vector.tensor_reduce(rmx[:, ci * 4 + ob: ci * 4 + ob + 1],
                                                        fpt[:], axis=AX.X, op=ALU.max)
                                nc.scalar.activation(face_h[:, ob * 1024 + c0: ob * 1024 + c0 + 512],
                                                     fpt[:], AF.Prelu,
                                                     bias=S["cfb"][:, ob: ob + 1], alpha=0.2)
                        # prelu is monotone: gml = prelu(max(raw) + cfb)
                        rmx2 = bp.tile([128, 4], f32, tag="rmx2", name="rmx2")
                        nc.vector.tensor_max(rmx2[:], rmx[:, 0:4], rmx[:, 4:8])
                        nc.vector.tensor_add(rmx2[:], rmx2[:], S["cfb"][:, 0:4])
                        nc.scalar.activation(gml[:], rmx2[:], AF.Prelu, alpha=0.2)
                        nc.sync.dma_start(gm_in[:], gml[:])
                        nc.gpsimd.collective_compute(
                            "AllReduce", ALU.max, replica_groups=GROUPS,
                            ins=[gm_in.opt()], outs=[gm_out.opt()],
                        )

                        # s1 matmuls on the face part run during the AllReduce;
                        # pre-bias results staged in SBUF until gb arrives
                        h2h = bp.tile([128, 4096], bf, tag="h2h", name="h2h")
                        uscr = bp.tile([128, 4096], f32, tag="uscr", name="uscr")
                        for ob in range(4):
                            for c0 in CHUNKS:
                                sp1 = psA.tile([128, 512], f32, tag="sp1", bufs=2, name="sp1")
                                for kbi in range(4):
                                    lh = S["s1fT"][:, kbi * 512 + 128 * ob: kbi * 512 + 128 * ob + 128]
                                    rh = face_h[:, kbi * 1024 + c0: kbi * 1024 + c0 + 512]
                                    nc.tensor.matmul(sp1[:], lh, rh, start=(kbi == 0), stop=(kbi == 3))
                                nc.vector.tensor_copy(uscr[:, ob * 1024 + c0: ob * 1024 + c0 + 512],
                                                      sp1[:])

                        gmg = bp.tile([128, 4], f32, tag="gmg", name="gmg")
                        nc.sync.dma_start(gmg[:], gm_out[:])
                        gmh = bp.tile([128, 4], bf, tag="gmh", name="gmh")
                        nc.vector.tensor_copy(gmh[:], gmg[:])

                        gb = bp.tile([128, 4], f32, tag="gb", name="gb")
                        for ob in range(4):
                            gvp = psA.tile([128, 1], f32, tag="gvp", bufs=2, name="gvp")
                            for kbi in range(4):
                                lh = S["s1gT"][:, kbi * 512 + 128 * ob: kbi * 512 + 128 * ob + 128]
                                rh = gmh[:, kbi: kbi + 1]
                                nc.tensor.matmul(gvp[:], lh, rh, start=(kbi == 0), stop=(kbi == 3))
                            nc.vector.tensor_add(gb[:, ob: ob + 1], gvp[:], S["s1b"][:, ob: ob + 1])

                        for ob in range(4):
                            for c0 in CHUNKS:
                                nc.scalar.activation(h2h[:, ob * 1024 + c0: ob * 1024 + c0 + 512],
                                                     uscr[:, ob * 1024 + c0: ob * 1024 + c0 + 512],
                                                     AF.Prelu, bias=gb[:, ob: ob + 1], alpha=0.2)

                    h3h = bp.tile([128, 2048], bf, tag="h3h", name="h3h")
                    outsb = bp.tile([50, 1024], f32, tag="outsb", name="outsb")

                    with tc.tile_pool(name="psB", bufs=1, space="PSUM") as psB:
                        for ob in range(2):
                            for c0 in CHUNKS:
                                sp2 = psB.tile([128, 512], f32, tag="sp2", bufs=2, name="sp2")
                                for kbi in range(4):
                                    lh = S["s2T"][:, kbi * 256 + 128 * ob: kbi * 256 + 128 * ob + 128]
                                    rh = h2h[:, kbi * 1024 + c0: kbi * 1024 + c0 + 512]
                                    nc.tensor.matmul(sp2[:], lh, rh, start=(kbi == 0), stop=(kbi == 3))
                                nc.scalar.activation(h3h[:, ob * 1024 + c0: ob * 1024 + c0 + 512],
                                                     sp2[:], AF.Prelu,
                                                     bias=S["s2b"][:, ob: ob + 1], alpha=0.2)

                        for c0 in CHUNKS:
                            sp3 = psB.tile([50, 512], f32, tag="sp3", bufs=2, name="sp3")
                            n = 0
                            for kbi in range(2):
                                lh = S["s3T"][:, kbi * 50: kbi * 50 + 50]
                                rh = h3h[:, kbi * 1024 + c0: kbi * 1024 + c0 + 512]
                                nc.tensor.matmul(sp3[:], lh, rh, start=(n == 0), stop=(n == 1))
                                n += 1
                            nc.vector.tensor_copy(outsb[:, c0: c0 + 512], sp3[:])

                    nc.sync.dma_start(out_d[:], outsb[:])

            for _ in range(reps):
                emit_pipeline()

    nc.compile()
    return nc


def _prep_shared(inputs):
    g = lambda k: np.asarray(inputs[k], np.float32)
    out = {}

    def fold(wn, gn, bn):
        return g(wn) * (INV * g(gn))[:, None], g(bn)

    def emit(nm, wf, dtype=BF16):
        wT = np.ascontiguousarray(wf.T)
        if wT.shape[0] > 128:
            wT = _blockP(wT)
        out[nm] = np.ascontiguousarray(wT.astype(dtype))

    w1, b1 = fold("conv1_w", "bn1_g", "bn1_b")
    w2, b2 = fold("conv2_w", "bn2_g", "bn2_b")
    w3, b3 = fold("conv3_w", "bn3_g", "bn3_b")
    wp1, bp1 = fold("pt1_w", "pt1_g", "pt1_b")
    wp2, bp2 = fold("pt2_w", "pt2_g", "pt2_b")
    emit("c1T", w1)
    emit("c2T", w2)
    emit("c3T", w3)
    emit("p1T", wp1)
    emit("p2T", wp2)
    fb1 = np.zeros((128, 1), np.float32)
    fb1[:64, 0] = b1
    out["fb1"] = fb1
    out["fb2"] = np.ascontiguousarray(b2[:, None])
    out["fb3"] = _blockP(b3[:, None]).astype(np.float32)
    out["fbp1"] = _blockP(bp1[:, None]).astype(np.float32)
    out["fbp2"] = _blockP(bp2[:, None]).astype(np.float32)

    for v, p in ((0, "sa1"), (1, "sa2")):
        emit(f"qkT{v}", g(p + "_qk"), np.float32)
        emit(f"vwT{v}", g(p + "_vw"))
        sg, sb2 = g(p + "_g"), g(p + "_b")
        twf = g(p + "_tw") * (INV * sg)[:, None]
        emit(f"twT{v}", twf)
        out[f"vbb{v}"] = np.ascontiguousarray(
            np.broadcast_to(g(p + "_vb")[None, :], (128, 256))).astype(np.float32)
        tbfv = g(p + "_tb") * (INV * sg) + sb2
        out[f"tbf{v}"] = _blockP(tbfv[:, None]).astype(np.float32)

    cfw, cfb_ = fold("cf_w", "cf_g", "cf_b")
    emit("cfT", cfw)
    out["cfb"] = _blockP(cfb_[:, None]).astype(np.float32)
    s1w, s1b_ = fold("s1_w", "s1_g", "s1_b")
    emit("s1fT", s1w[:, :512])
    emit("s1gT", s1w[:, 512:])
    out["s1b"] = _blockP(s1b_[:, None]).astype(np.float32)
    s2w, s2b_ = fold("s2_w", "s2_g", "s2_b")
    emit("s2T", s2w)
    out["s2b"] = _blockP(s2b_[:, None]).astype(np.float32)
    emit("s3T", g("s3_w"))

    for nm, (sh, dn) in SPECS.items():
        if nm.startswith("xT"):
            continue
        a = out[nm]
        assert tuple(a.shape) == sh, (nm, a.shape, sh)
        assert (a.dtype == BF16) == (dn == "bf"), (nm, a.dtype)
    return out


def _get_nc(reps=1):
    key = "nc" + str(reps)
    if key not in _CACHE:
        _CACHE[key] = _build(reps)
    return _CACHE[key]


def _hilo(a):
    a = np.ascontiguousarray(np.asarray(a, dtype=np.float32))
    hi = a.astype(BF16)
    lo = (a - hi.astype(np.float32)).astype(BF16)
    return hi, lo


def _in_maps(inputs):
    base = _prep_shared(inputs)
    x = np.asarray(inputs["x"], np.float32)
    maps = []
    for c in range(8):
        b, j = c // 4, c % 4
        xT = np.ascontiguousarray(x[b, 1024 * j: 1024 * (j + 1), :].T)
        hi, lo = _hilo(xT)
        m = dict(base)
        m["xT_hi"], m["xT_lo"] = hi, lo
        maps.append(m)
    return maps


def _assemble(results):
    full = np.empty((2, 4096, 50), np.float32)
    for c in range(8):
        b, j = c // 4, c % 4
        full[b, 1024 * j: 1024 * (j + 1), :] = np.asarray(results[c]["out"], np.float32).T
    return full


def _run_preput(nc, in_maps):
    """Execute the prebuilt Bass module on 8 cores via one sharded PJRT call,
    with all inputs pre-placed on device so every core launches together
    (otherwise per-core H2D transfer skew is absorbed into the first
    collective wait on the early cores)."""
    import jax
    from jax.sharding import Mesh, PartitionSpec, NamedSharding
    from jax.experimental.shard_map import shard_map
    from concourse import mybir
    from concourse.bass2jax import (
        _bass_exec_p, install_neuronx_cc_hook, partition_id_tensor)

    install_neuronx_cc_hook()
    partition_name = nc.partition_id_tensor.name if nc.partition_id_tensor else None
    in_names, out_names, out_avals, zero_outs = [], [], [], []
    for alloc in nc.m.functions[0].allocations:
        if not isinstance(alloc, mybir.MemoryLocationSet):
            continue
        name = alloc.memorylocations[0].name
        if alloc.kind == "ExternalInput":
            if name != partition_name:
                in_names.append(name)
        elif alloc.kind == "ExternalOutput":
            out_names.append(name)
            shape = tuple(alloc.tensor_shape)
            dtype = mybir.dt.np(alloc.dtype)
            out_avals.append(jax.core.ShapedArray(shape, dtype))
            zero_outs.append(np.zeros(shape, dtype))
    n_params = len(in_names)
    in_names_all = in_names + out_names
    if partition_name is not None:
        in_names_all.append(partition_name)

    def _body(*args):
        operands = list(args)
        if partition_name is not None:
            operands.append(partition_id_tensor())
        outs = _bass_exec_p.bind(
            *operands, out_avals=tuple(out_avals), in_names=tuple(in_names_all),
            out_names=tuple(out_names), lowering_input_output_aliases=(),
            sim_require_finite=True, sim_require_nnan=True, nc=nc)
        return tuple(outs)

    devices = jax.devices()[:8]
    mesh = Mesh(np.asarray(devices), ("core",))
    spec = PartitionSpec("core")
    fn = jax.jit(
        shard_map(_body, mesh=mesh, in_specs=(spec,) * (n_params + len(out_avals)),
                  out_specs=(spec,) * len(out_avals), check_rep=False),
        keep_unused=True)
    per_core = [[np.asarray(m[name]) for name in in_names] for m in in_maps]
    concat_in = [np.concatenate([per_core[c][i] for c in range(8)], axis=0)
                 for i in range(n_params)]
    concat_zeros = [np.zeros((8 * zz.shape[0], *zz.shape[1:]), zz.dtype)
                    for zz in zero_outs]
    sh = NamedSharding(mesh, spec)
    dev_in = [jax.device_put(a, sh) for a in concat_in]
    dev_zero = [jax.device_put(a, sh) for a in concat_zeros]
    jax.block_until_ready(dev_in)
    jax.block_until_ready(dev_zero)
    # Compile before the timed/traced execution so tracing+XLA compile
    # don't sit between device placement and launch.
    fn_c = fn.lower(*dev_in, *dev_zero).compile()
    out_arrs = fn_c(*dev_in, *dev_zero)
    jax.block_until_ready(out_arrs)
    return [
        {name: np.asarray(out_arrs[i]).reshape(8, *out_avals[i].shape)[c]
         for i, name in enumerate(out_names)}
        for c in range(8)
    ]


def kernel(**inputs):
    nc = _get_nc()
    results = _run_preput(nc, _in_maps(inputs))
    return _assemble(results)


def measure_hw_ns(inputs, M=64, reps=1):
    import time
    import jax
    from jax.sharding import Mesh, PartitionSpec, NamedSharding
    from jax.experimental.shard_map import shard_map
    from concourse import mybir
    from concourse.bass2jax import _bass_exec_p, install_neuronx_cc_hook, partition_id_tensor

    nc = _get_nc(reps=reps)
    install_neuronx_cc_hook()
    in_maps = _in_maps(inputs)
    partition_name = nc.partition_id_tensor.name if nc.partition_id_tensor else None
    in_names, out_names, out_avals, zero_outs = [], [], [], []
    for alloc in nc.m.functions[0].allocations:
        if not isinstance(alloc, mybir.MemoryLocationSet):
            continue
        name = alloc.memorylocations[0].name
        if alloc.kind == "ExternalInput":
            if name != partition_name:
                in_names.append(name)
        elif alloc.kind == "ExternalOutput":
            out_names.append(name)
            shape = tuple(alloc.tensor_shape)
            dtype = mybir.dt.np(alloc.dtype)
            out_avals.append(jax.core.ShapedArray(shape, dtype))
            zero_outs.append(np.zeros(shape, dtype))
    n_params = len(in_names)
    in_names_all = in_names + out_names
    if partition_name is not None:
        in_names_all.append(partition_name)

    def _body(*args):
        operands = list(args)
        if partition_name is not None:
            operands.append(partition_id_tensor())
        outs = _bass_exec_p.bind(
            *operands, out_avals=tuple(out_avals), in_names=tuple(in_names_all),
            out_names=tuple(out_names), lowering_input_output_aliases=(),
            sim_require_finite=True, sim_require_nnan=True, nc=nc)
        return tuple(outs)

    devices = jax.devices()[:8]
    mesh = Mesh(np.asarray(devices), ("core",))
    spec = PartitionSpec("core")
    fn = jax.jit(
        shard_map(_body, mesh=mesh, in_specs=(spec,) * (n_params + len(out_avals)),
                  out_specs=(spec,) * len(out_avals), check_rep=False),
        keep_unused=True)
    per_core = [[np.asarray(m[name]) for name in in_names] for m in in_maps]
    concat_in = [np.concatenate([per_core[c][i] for c in range(8)], axis=0)
                 for i in range(n_params)]
    concat_zeros = [np.zeros((8 * zz.shape[0], *zz.shape[1:]), zz.dtype) for zz in zero_outs]
    sh = NamedSharding(mesh, spec)
    dev_in = [jax.device_put(a, sh) for a in concat_in]
    dev_zero = [jax.device_put(a, sh) for a in concat_zeros]
    o = fn(*dev_in, *dev_zero)
    jax.block_until_ready(o)
    t0 = time.perf_counter()
    outs = [fn(*dev_in, *dev_zero) for _ in range(M)]
    jax.block_until_ready(outs)
    t1 = time.perf_counter()
    return (t1 - t0) / M * 1e9
